# revision 18
# baseline (speedup 1.0000x reference)
"""MACCL loss kernel for Trainium2 (8 NeuronCores, SPMD data-parallel).

Strategy (v4: transposed j-blocks, PE-side reduction)
-----------------------------------------------------
The O(B^2) contrastive exp/row-sum dominates.  v2 ran it all on the
Scalar (ACT) engine (~67us busy); v3's ACT/DVE column split stalled on
convoy effects (DVE needs 2 passes row-major, PE ran cold).  v4 flips
the orientation: each core computes sim^T in 64 j-blocks of 128
columns; per block the stationary operand is the a8 j-block and the
moving operand is the core's own 1024 rows, so psum holds [128 j-rows,
1024 i-cols].

  - Even blocks -> ACT: true exp, activation(psum, Exp, bias=-7ln2),
    output straight to fp8(e5m2) scratch (exp(q)*2^-7), NO accumulator.
  - Odd blocks -> DVE: Schraudolph fast exp in ONE 1x pass:
    n = int8(psum*C1E + C2E); the int8 bits ARE the e5m2 encoding of
    exp(q)*2^-7.
  - The row sums (pos/neg by label) are done by the PE itself: per
    block pair two tiny fp8 DoubleRow matmuls with a [256 x 32] label
    selector (only cols 0/1 nonzero; labels ride in as DATA, so one
    program serves any n0) accumulate S0/S1 for all 1024 rows into two
    PSUM banks (one per 512-row i-half; DR dst must sit at the col_grp
    base partition).  Dummy matmuls during the DMA lead-in pre-warm the
    PE's HAM clock gate.

All engines stream continuously (PE stays HAM-warm), there are no
ACT read-accumulator ops, and the only DVE work is one 1x pass per odd
block.  Host subtracts the huge exp(diag) (~1.6e6 vs pos_sum ~6.5e3)
with a bit-exact f64 replication of whichever engine produced that
column (np.exp+e5m2 rounding for ACT blocks, the int8 Schraudolph for
DVE blocks); a safety net recomputes exactly any row whose pos/neg
sums look corrupted (rare fp8 rounding flips / low-tail underflow), so
numerics are robust to convert rounding modes.

Host prep (O(B*D), f64): label-sort rows, row norms/sums for the
center/margin/sigma terms, fp8(e4m3) quantization of both operands
(stationary carries r_j, moving carries r_i/T).
"""

import os
import sys

for _p in ("/root/.axon_site", "/root/.axon_site/_ro/trn_rl_repo",
           "/root/.axon_site/_ro/pypackages", "/opt/trn_rl_repo", "/opt/pypackages"):
    if os.path.isdir(_p) and _p not in sys.path:
        sys.path.append(_p)

import numpy as np
import ml_dtypes
from contextlib import ExitStack

import concourse.bass as bass
import concourse.bacc as bacc
import concourse.tile as tile
from concourse import mybir
from concourse.bass_utils import run_bass_kernel_spmd

F32 = mybir.dt.float32
I8 = mybir.dt.int8
F8 = mybir.dt.float8e4
F8E5 = mybir.dt.float8e5

P = 128
D = 256
B = 8192
NCORES = 8
BPC = B // NCORES          # 1024 rows per core
NBLK = B // P              # 64 j-blocks
NPAIR = NBLK // 2          # 32 block pairs
N_MM = 512
TEMPERATURE = 0.07
MARGIN_BASE = 0.5
LAMBDA_SIGMA = 0.3
LAMBDA_RESOLUTION = 0.3
RESOLUTION_RATIO = 224.0 / 900.0
ALPHA, BETA, GAMMA = 1.0, 1.0, 0.5

LOG2 = float(np.log(2.0))
SCALE_LOG = 7                      # scratch holds exp(q) * 2^-SCALE_LOG
EBIAS = float(np.float32(-SCALE_LOG * LOG2))
# e5m2/int8 Schraudolph: n = rint(q*C1E + C2E); bitcast_e5m2(n) ~ exp(q)*2^-7
C1E = float(np.float32(4.0 / LOG2))
C2E = float(np.float32(32.0 - 0.22))


def build_program():
    AF = mybir.ActivationFunctionType
    ALU = mybir.AluOpType
    DR = mybir.MatmulPerfMode.DoubleRow

    nc = bacc.Bacc("TRN2", target_bir_lowering=False, debug=False,
                   num_devices=NCORES)
    a8_d = nc.dram_tensor("a8", [P, 2, B], F8, kind="ExternalInput").ap()
    mm8_d = nc.dram_tensor("mm8", [P, 2, BPC], F8, kind="ExternalInput").ap()
    sel_d = nc.dram_tensor("sel", [P, NPAIR, 2, 32], F8E5,
                           kind="ExternalInput").ap()
    stats_d = nc.dram_tensor("stats", [2, BPC], F32, kind="ExternalOutput").ap()

    with tile.TileContext(nc) as tc, ExitStack() as ctx:
        singles = ctx.enter_context(tc.tile_pool(name="singles", bufs=1))
        ps_pool = ctx.enter_context(tc.tile_pool(name="ps", bufs=1, space="PSUM"))

        a8_sb = singles.tile([P, 2, B], F8)
        m8_sb = singles.tile([P, 2, BPC], F8)
        sel_sb = singles.tile([P, NPAIR, 2, 32], F8E5)
        scrs = [singles.tile([P, 2, BPC], F8E5, name=f"scr{t}")
                for t in range(3)]
        stats_sb = singles.tile([P, BPC], F32)
        prime = singles.tile([P, 1], F32)
        ebias_t = singles.tile([P, 1], F32)
        nc.vector.memset(ebias_t, EBIAS)

        # Priming activation with no input deps (scale=0 ignores the garbage
        # read): hoists the ~1.5us ACT table load into the DMA lead-in.
        nc.scalar.activation(prime, prime, AF.Exp, scale=0.0)

        # moving operand + selectors first (gate everything), then a8 with a
        # fine-grained head so the first matmuls start early.
        nc.sync.dma_start(m8_sb[:, :, 0:N_MM], mm8_d[:, :, 0:N_MM])
        nc.sync.dma_start(a8_sb[:, :, 0:P], a8_d[:, :, 0:P])
        nc.sync.dma_start(m8_sb[:, :, N_MM:BPC], mm8_d[:, :, N_MM:BPC])
        # lower half of a8 on the sync queue (feeds the first ~32 blocks);
        # upper half + selectors in parallel on the gpsimd hwdge queue.
        cuts = [128, 256, 512, 1024, 2048, 3072, 4096]
        for c0, c1 in zip(cuts[:-1], cuts[1:]):
            nc.sync.dma_start(a8_sb[:, :, c0:c1], a8_d[:, :, c0:c1])
        for c0, c1 in ((4096, 5120), (5120, 6144), (6144, 7168), (7168, B)):
            nc.gpsimd.dma_start(a8_sb[:, :, c0:c1], a8_d[:, :, c0:c1])
        nc.gpsimd.dma_start(sel_sb, sel_d)

        Ts = [ps_pool.tile([P, BPC], F32, name=f"T{t}") for t in range(3)]
        accs = [ps_pool.tile([P, N_MM], F32, name=f"acc{h}") for h in (0, 1)]

        # HAM warm-up: keep the PE busy during the DMA lead-in (on the first
        # m8 chunk, which lands early) so the 4096-cycle activity window
        # un-throttles the clock before the real matmul stream begins.
        for w in range(14):
            nc.tensor.matmul(Ts[2][:, 0:64], m8_sb[:, :, 0:P],
                             m8_sb[:, :, 0:64], start=True, stop=True,
                             perf_mode=DR)

        def emit_sel(q):
            sc = scrs[q % 3]
            for h in (0, 1):
                nc.tensor.matmul(accs[h][0:32, :],
                                 sel_sb[:, q, :, :],
                                 sc[:, :, h * N_MM:(h + 1) * N_MM],
                                 start=(q == 0), stop=(q == NPAIR - 1),
                                 perf_mode=DR, skip_group_check=True)

        for b in range(NBLK):
            q, half = divmod(b, 2)
            if half == 0 and q >= 2:
                emit_sel(q - 2)
            T = Ts[b % 3]
            lhsT = a8_sb[:, :, b * P:(b + 1) * P]
            for h in (0, 1):
                nc.tensor.matmul(T[:, h * N_MM:(h + 1) * N_MM], lhsT,
                                 m8_sb[:, :, h * N_MM:(h + 1) * N_MM],
                                 start=True, stop=True, perf_mode=DR)
            sc = scrs[q % 3]
            if half == 0:
                nc.scalar.activation(sc[:, 0, :], T, AF.Exp,
                                     bias=ebias_t[:, 0:1])
            else:
                nc.vector.tensor_scalar(sc.bitcast(I8)[:, 1, :], T,
                                        C1E, C2E, ALU.mult, ALU.add)
        emit_sel(NPAIR - 2)
        emit_sel(NPAIR - 1)

        # acc_h rows 0/1 = S0/S1 for i-half h.
        nc.vector.tensor_copy(stats_sb[0:2, 0:N_MM], accs[0][0:2, :])
        nc.scalar.copy(stats_sb[0:2, N_MM:BPC], accs[1][0:2, :])
        nc.sync.dma_start(stats_d[:, 0:N_MM], stats_sb[0:2, 0:N_MM])
        nc.sync.dma_start(stats_d[:, N_MM:BPC], stats_sb[0:2, N_MM:BPC])

    nc.compile()
    return nc


_PROGRAM_CACHE = {}


def _get_program():
    if "p" not in _PROGRAM_CACHE:
        _PROGRAM_CACHE["p"] = build_program()
    return _PROGRAM_CACHE["p"]


def _schraud_f64(q):
    """f64 replication of the device DVE fast exp (RNE convert assumed),
    including the 2^SCALE_LOG unscale."""
    y1 = (q.astype(np.float32) * np.float32(C1E)).astype(np.float32)
    y2 = (y1 + np.float32(C2E)).astype(np.float32)
    n = np.rint(y2.astype(np.float64)).astype(np.int8)
    return n.view(ml_dtypes.float8_e5m2).astype(np.float64) * (2.0 ** SCALE_LOG)


def _actexp_f64(q):
    """f64 replication of the ACT-block diag: exp(q-7ln2) rounded to e5m2."""
    v = np.exp(q - SCALE_LOG * LOG2).astype(np.float32)
    v8 = v.astype(ml_dtypes.float8_e5m2)
    return v8.astype(np.float64) * (2.0 ** SCALE_LOG)


def run_device(features, labels, trace=False):
    """Host prep + 8-core device run.  Returns (stats dict aligned to the
    label-sorted permutation, permutation order, n0, raw results)."""
    Bq, d = features.shape
    assert d == D and Bq == B

    order = np.argsort(labels, kind="stable")
    n0 = int((labels == 0).sum())
    fp = np.ascontiguousarray(features[order]).astype(np.float32, copy=False)

    # host-side O(B*D) prep
    fp64 = fp.astype(np.float64)
    norms2 = (fp64 * fp64).sum(axis=1)                  # [B]
    rowsum = fp64.sum(axis=1)                           # [B]
    r = 1.0 / np.maximum(np.sqrt(norms2), 1e-12)        # [B]
    r32 = r.astype(np.float32)

    # [K=128, 2, B] DoubleRow layout: D index = ktile*128 + partition.
    # mm8 (moving) carries r_i/T so psum holds sim/T directly.
    fT = np.ascontiguousarray(fp.T).reshape(2, P, B).transpose(1, 0, 2)
    sT = (r32 / np.float32(TEMPERATURE)).astype(np.float32)
    m8_full = np.ascontiguousarray(fT * sT[None, None, :]).astype(
        ml_dtypes.float8_e4m3)
    a8 = np.ascontiguousarray(fT * r32[None, None, :]).astype(
        ml_dtypes.float8_e4m3)

    # label selector: sel[p, q, kt, c] = 1 iff label(j=256q+128kt+p) == c
    lab = (np.arange(B) >= n0)
    labq = lab.reshape(NPAIR, 2, P)
    sel = np.zeros((P, NPAIR, 2, 32), dtype=ml_dtypes.float8_e5m2)
    sel[:, :, :, 0] = (~labq).transpose(2, 0, 1).astype(ml_dtypes.float8_e5m2)
    sel[:, :, :, 1] = labq.transpose(2, 0, 1).astype(ml_dtypes.float8_e5m2)

    nc = _get_program()
    in_maps = []
    for c in range(NCORES):
        sl = slice(c * BPC, (c + 1) * BPC)
        in_maps.append({"a8": a8, "sel": sel,
                        "mm8": np.ascontiguousarray(m8_full[:, :, sl])})
    res = run_bass_kernel_spmd(nc, in_maps, list(range(NCORES)), trace=trace)

    parts = [res.results[c]["stats"].astype(np.float64) for c in range(NCORES)]
    S0 = np.concatenate([p[0] for p in parts]) * (2.0 ** SCALE_LOG)
    S1 = np.concatenate([p[1] for p in parts]) * (2.0 ** SCALE_LOG)

    # Diagonal exp reproduction: q'[i] = the device's own quantized
    # self-product (f64); formula matched to the engine of block i//128.
    q = np.einsum("pkj,pkj->j", m8_full.astype(np.float64),
                  a8.astype(np.float64))
    act_block = ((np.arange(B) // P) % 2) == 0
    dd = np.where(act_block, _actexp_f64(q), _schraud_f64(q))

    fn64 = fp64 * r[:, None]
    stats = {"norms2": norms2, "rowsum": rowsum, "S0": S0, "S1": S1,
             "d": dd, "fn64": fn64}
    return stats, order, n0, res


def _contrastive(stats, order, n0, labels, B):
    """Per-row r_con (f64) from device sums + host safety net."""
    labels_p = labels[order]
    nmf = (labels_p == 0)
    S0 = stats["S0"]
    S1 = stats["S1"]
    ddiag = stats["d"]

    S_same = np.where(nmf, S0, S1)
    S_diff = np.where(nmf, S1, S0)
    pos_sum = S_same - ddiag
    neg_sum = S_diff.copy()
    n1 = B - n0
    cnt_pos = np.where(nmf, n0 - 1, n1 - 1)
    cnt_neg = np.where(nmf, n1, n0)
    has_both = (cnt_pos > 0) & (cnt_neg > 0)

    if has_both.any():
        hb_pos = pos_sum[has_both]
        med = np.median(hb_pos[np.isfinite(hb_pos)]) if np.isfinite(
            hb_pos).any() else 0.0
        thresh = 0.25 * max(med, 0.0)
        suspect = has_both & (
            (pos_sum < thresh) | ~np.isfinite(pos_sum)
            | (neg_sum <= 0.0) | ~np.isfinite(neg_sum))
        idx = np.nonzero(suspect)[0]
        if idx.size:
            fn = stats["fn64"]
            for lo in range(0, idx.size, 512):
                rows = idx[lo:lo + 512]
                sims = (fn[rows] @ fn.T) / TEMPERATURE
                e = np.exp(sims)
                same = labels_p[rows][:, None] == labels_p[None, :]
                e_same = np.where(same, e, 0.0).sum(axis=1)
                e_diff = np.where(same, 0.0, e).sum(axis=1)
                self_e = np.exp(sims[np.arange(rows.size), rows])
                pos_sum[rows] = e_same - self_e
                neg_sum[rows] = e_diff

    pos_safe = np.where(has_both, np.maximum(pos_sum, 1e-12), 1.0)
    den_safe = np.where(has_both, pos_sum + neg_sum + 1e-8, 1.0)
    return np.where(has_both, -np.log(pos_safe / den_safe), 0.0), labels_p


def finalize(stats, order, n0, labels, normal_center, running_sigma, B):
    """Host O(B) finalization mirroring the reference formulas (float64)."""
    r_con, labels_p = _contrastive(stats, order, n0, labels, B)
    nmf = (labels_p == 0)
    amf = (labels_p == 1)
    norms2 = stats["norms2"]
    rowsum = stats["rowsum"]

    c = np.asarray(normal_center, dtype=np.float64)
    csq = float((c * c).sum())
    if csq != 0.0:
        raise NotImplementedError  # caller routes to the general-center path
    dist_sq = norms2  # center == 0
    n_normal = float(nmf.sum())

    with np.errstate(divide="ignore", invalid="ignore"):
        n_el = n_normal * D
        masked_sum = float((rowsum * nmf).sum())
        mean = masked_sum / n_el
        sum_sq_m = float((norms2 * nmf).sum())
        var = (sum_sq_m - 2.0 * mean * masked_sum + mean * mean * n_el) / (n_el - 1.0)
        sigma_new = 0.9 * float(running_sigma) + 0.1 * np.sqrt(var)

        m_adaptive = (MARGIN_BASE + LAMBDA_SIGMA * sigma_new
                      + LAMBDA_RESOLUTION * (1.0 - RESOLUTION_RATIO))
        dist = np.sqrt(np.maximum(dist_sq, 0.0))
        r_center = dist_sq * nmf
        r_margin = np.maximum(m_adaptive - dist, 0.0) * amf

        raw_total = ALPHA * r_center + BETA * r_margin + GAMMA * r_con
        total = raw_total.mean()
    return np.array(total, dtype=np.float32)


def _finalize_general_center(stats, order, n0, labels, normal_center,
                             running_sigma, B, features):
    """Fallback for a nonzero normal_center (not hit for spec inputs)."""
    r_con, labels_p = _contrastive(stats, order, n0, labels, B)
    fp = features[order].astype(np.float64)
    c = np.asarray(normal_center, dtype=np.float64)
    qc = fp @ c
    norms2 = stats["norms2"]
    dist_sq = norms2 - 2.0 * qc + float((c * c).sum())
    nmf = (labels_p == 0)
    amf = (labels_p == 1)
    rowsum = stats["rowsum"]
    n_normal = float(nmf.sum())
    with np.errstate(divide="ignore", invalid="ignore"):
        n_el = n_normal * D
        masked_sum = float((rowsum * nmf).sum())
        mean = masked_sum / n_el
        sum_sq_m = float((norms2 * nmf).sum())
        var = (sum_sq_m - 2.0 * mean * masked_sum + mean * mean * n_el) / (n_el - 1.0)
        sigma_new = 0.9 * float(running_sigma) + 0.1 * np.sqrt(var)
        m_adaptive = (MARGIN_BASE + LAMBDA_SIGMA * sigma_new
                      + LAMBDA_RESOLUTION * (1.0 - RESOLUTION_RATIO))
        dist = np.sqrt(np.maximum(dist_sq, 0.0))
        r_center = dist_sq * nmf
        r_margin = np.maximum(m_adaptive - dist, 0.0) * amf
        total = (ALPHA * r_center + BETA * r_margin + GAMMA * r_con).mean()
    return np.array(total, dtype=np.float32)


def kernel(features, labels, normal_center, running_sigma):
    features = np.asarray(features, dtype=np.float32)
    labels = np.asarray(labels, dtype=np.int32)
    normal_center = np.asarray(normal_center, dtype=np.float32)
    running_sigma = np.float32(np.asarray(running_sigma))
    Bq = features.shape[0]

    stats, order, n0, _res = run_device(features, labels)
    if float((np.asarray(normal_center, np.float64) ** 2).sum()) != 0.0:
        return _finalize_general_center(stats, order, n0, labels,
                                        normal_center, running_sigma, Bq,
                                        features)
    return finalize(stats, order, n0, labels, normal_center, running_sigma, Bq)


# revision 19
# speedup vs baseline: 1.0467x; 1.0467x over previous
"""MACCL loss kernel for Trainium2 (8 NeuronCores, SPMD data-parallel).

Strategy (v4: transposed j-blocks, PE-side reduction)
-----------------------------------------------------
The O(B^2) contrastive exp/row-sum dominates.  v2 ran it all on the
Scalar (ACT) engine (~67us busy); v3's ACT/DVE column split stalled on
convoy effects (DVE needs 2 passes row-major, PE ran cold).  v4 flips
the orientation: each core computes sim^T in 64 j-blocks of 128
columns; per block the stationary operand is the a8 j-block and the
moving operand is the core's own 1024 rows, so psum holds [128 j-rows,
1024 i-cols].

  - Even blocks -> ACT: true exp, activation(psum, Exp, bias=-7ln2),
    output straight to fp8(e5m2) scratch (exp(q)*2^-7), NO accumulator.
  - Odd blocks -> DVE: Schraudolph fast exp in ONE 1x pass:
    n = int8(psum*C1E + C2E); the int8 bits ARE the e5m2 encoding of
    exp(q)*2^-7.
  - The row sums (pos/neg by label) are done by the PE itself: per
    block pair two tiny fp8 DoubleRow matmuls with a [256 x 32] label
    selector (only cols 0/1 nonzero; labels ride in as DATA, so one
    program serves any n0) accumulate S0/S1 for all 1024 rows into two
    PSUM banks (one per 512-row i-half; DR dst must sit at the col_grp
    base partition).  Dummy matmuls during the DMA lead-in pre-warm the
    PE's HAM clock gate.

All engines stream continuously (PE stays HAM-warm), there are no
ACT read-accumulator ops, and the only DVE work is one 1x pass per odd
block.  Host subtracts the huge exp(diag) (~1.6e6 vs pos_sum ~6.5e3)
with a bit-exact f64 replication of whichever engine produced that
column (np.exp+e5m2 rounding for ACT blocks, the int8 Schraudolph for
DVE blocks); a safety net recomputes exactly any row whose pos/neg
sums look corrupted (rare fp8 rounding flips / low-tail underflow), so
numerics are robust to convert rounding modes.

Host prep (O(B*D), f64): label-sort rows, row norms/sums for the
center/margin/sigma terms, fp8(e4m3) quantization of both operands
(stationary carries r_j, moving carries r_i/T).
"""

import os
import sys

for _p in ("/root/.axon_site", "/root/.axon_site/_ro/trn_rl_repo",
           "/root/.axon_site/_ro/pypackages", "/opt/trn_rl_repo", "/opt/pypackages"):
    if os.path.isdir(_p) and _p not in sys.path:
        sys.path.append(_p)

import numpy as np
import ml_dtypes
from contextlib import ExitStack

import concourse.bass as bass
import concourse.bacc as bacc
import concourse.tile as tile
from concourse import mybir
from concourse.bass_utils import run_bass_kernel_spmd

F32 = mybir.dt.float32
I8 = mybir.dt.int8
F8 = mybir.dt.float8e4
F8E5 = mybir.dt.float8e5

P = 128
D = 256
B = 8192
NCORES = 8
BPC = B // NCORES          # 1024 rows per core
NBLK = B // P              # 64 j-blocks
NPAIR = NBLK // 2          # 32 block pairs
N_MM = 512
TEMPERATURE = 0.07
MARGIN_BASE = 0.5
LAMBDA_SIGMA = 0.3
LAMBDA_RESOLUTION = 0.3
RESOLUTION_RATIO = 224.0 / 900.0
ALPHA, BETA, GAMMA = 1.0, 1.0, 0.5

LOG2 = float(np.log(2.0))
SCALE_LOG = 7                      # scratch holds exp(q) * 2^-SCALE_LOG
EBIAS = float(np.float32(-SCALE_LOG * LOG2))
# e5m2/int8 Schraudolph: n = rint(q*C1E + C2E); bitcast_e5m2(n) ~ exp(q)*2^-7
C1E = float(np.float32(4.0 / LOG2))
C2E = float(np.float32(32.0 - 0.22))


def build_program():
    AF = mybir.ActivationFunctionType
    ALU = mybir.AluOpType
    DR = mybir.MatmulPerfMode.DoubleRow
    DRSW = mybir.MatmulPerfMode.DoubleRowSwInterleave

    nc = bacc.Bacc("TRN2", target_bir_lowering=False, debug=False,
                   num_devices=NCORES)
    a8_d = nc.dram_tensor("a8", [P, NBLK, 256], F8, kind="ExternalInput").ap()
    mm8_d = nc.dram_tensor("mm8", [P, 2, BPC], F8, kind="ExternalInput").ap()
    sel_d = nc.dram_tensor("sel", [P, NPAIR, 2, 32], F8E5,
                           kind="ExternalInput").ap()
    stats_d = nc.dram_tensor("stats", [2, BPC], F32, kind="ExternalOutput").ap()

    with tile.TileContext(nc) as tc, ExitStack() as ctx:
        singles = ctx.enter_context(tc.tile_pool(name="singles", bufs=1))
        ps_pool = ctx.enter_context(tc.tile_pool(name="ps", bufs=1, space="PSUM"))

        a8_sb = singles.tile([P, NBLK, 256], F8)
        m8_sb = singles.tile([P, 2, BPC], F8)
        sel_sb = singles.tile([P, NPAIR, 2, 32], F8E5)
        scrs = [singles.tile([P, 2, BPC], F8E5, name=f"scr{t}")
                for t in range(3)]
        stats_sb = singles.tile([P, BPC], F32)
        prime = singles.tile([P, 1], F32)
        ebias_t = singles.tile([P, 1], F32)
        nc.vector.memset(ebias_t, EBIAS)

        # Priming activation with no input deps (scale=0 ignores the garbage
        # read): hoists the ~1.5us ACT table load into the DMA lead-in.
        nc.scalar.activation(prime, prime, AF.Exp, scale=0.0)

        # moving operand + selectors first (gate everything), then a8 with a
        # fine-grained head so the first matmuls start early.
        nc.sync.dma_start(m8_sb[:, :, 0:N_MM], mm8_d[:, :, 0:N_MM])
        nc.sync.dma_start(a8_sb[:, 0:1], a8_d[:, 0:1])
        nc.sync.dma_start(m8_sb[:, :, N_MM:BPC], mm8_d[:, :, N_MM:BPC])
        # a8 blocks stream on the sync queue; the selector DMA is placed
        # late enough not to stall the first blocks but well before the PE
        # reaches the first selector matmul.
        cuts = [1, 2, 4, 8, 16, 24, 32, 40, 48, 56, NBLK]
        for ci, (c0, c1) in enumerate(zip(cuts[:-1], cuts[1:])):
            nc.sync.dma_start(a8_sb[:, c0:c1], a8_d[:, c0:c1])
            if ci == 3:
                nc.sync.dma_start(sel_sb, sel_d)

        Ts = [ps_pool.tile([P, BPC], F32, name=f"T{t}") for t in range(3)]
        accs = [ps_pool.tile([P, N_MM], F32, name=f"acc{h}") for h in (0, 1)]

        # HAM warm-up: keep the PE busy during the DMA lead-in (on the first
        # m8 chunk, which lands early) so the 4096-cycle activity window
        # un-throttles the clock before the real matmul stream begins.
        for w in range(14):
            nc.tensor.matmul(Ts[2][:, 0:64], m8_sb[:, :, 0:P],
                             m8_sb[:, :, 0:64], start=True, stop=True,
                             perf_mode=DR)

        def emit_sel(q):
            sc = scrs[q % 3]
            for h in (0, 1):
                nc.tensor.matmul(accs[h][0:32, :],
                                 sel_sb[:, q, :, :],
                                 sc[:, :, h * N_MM:(h + 1) * N_MM],
                                 start=(q == 0), stop=(q == NPAIR - 1),
                                 perf_mode=DR, skip_group_check=True)

        for b in range(NBLK):
            q, half = divmod(b, 2)
            if half == 0 and q >= 2:
                emit_sel(q - 2)
            T = Ts[b % 3]
            lhsT = a8_sb[:, b, :]
            for h in (0, 1):
                nc.tensor.matmul(T[:, h * N_MM:(h + 1) * N_MM], lhsT,
                                 m8_sb[:, :, h * N_MM:(h + 1) * N_MM],
                                 start=True, stop=True, perf_mode=DRSW)
            sc = scrs[q % 3]
            if half == 0:
                nc.scalar.activation(sc[:, 0, :], T, AF.Exp,
                                     bias=ebias_t[:, 0:1])
            else:
                nc.vector.tensor_scalar(sc.bitcast(I8)[:, 1, :], T,
                                        C1E, C2E, ALU.mult, ALU.add)
        emit_sel(NPAIR - 2)
        emit_sel(NPAIR - 1)

        # acc_h rows 0/1 = S0/S1 for i-half h.
        nc.vector.tensor_copy(stats_sb[0:2, 0:N_MM], accs[0][0:2, :])
        nc.scalar.copy(stats_sb[0:2, N_MM:BPC], accs[1][0:2, :])
        nc.sync.dma_start(stats_d[:, 0:N_MM], stats_sb[0:2, 0:N_MM])
        nc.sync.dma_start(stats_d[:, N_MM:BPC], stats_sb[0:2, N_MM:BPC])

    nc.compile()
    return nc


_PROGRAM_CACHE = {}


def _get_program():
    if "p" not in _PROGRAM_CACHE:
        _PROGRAM_CACHE["p"] = build_program()
    return _PROGRAM_CACHE["p"]


def _schraud_f64(q):
    """f64 replication of the device DVE fast exp (RNE convert assumed),
    including the 2^SCALE_LOG unscale."""
    y1 = (q.astype(np.float32) * np.float32(C1E)).astype(np.float32)
    y2 = (y1 + np.float32(C2E)).astype(np.float32)
    n = np.rint(y2.astype(np.float64)).astype(np.int8)
    return n.view(ml_dtypes.float8_e5m2).astype(np.float64) * (2.0 ** SCALE_LOG)


def _actexp_f64(q):
    """f64 replication of the ACT-block diag: exp(q-7ln2) rounded to e5m2."""
    v = np.exp(q - SCALE_LOG * LOG2).astype(np.float32)
    v8 = v.astype(ml_dtypes.float8_e5m2)
    return v8.astype(np.float64) * (2.0 ** SCALE_LOG)


def run_device(features, labels, trace=False):
    """Host prep + 8-core device run.  Returns (stats dict aligned to the
    label-sorted permutation, permutation order, n0, raw results)."""
    Bq, d = features.shape
    assert d == D and Bq == B

    order = np.argsort(labels, kind="stable")
    n0 = int((labels == 0).sum())
    fp = np.ascontiguousarray(features[order]).astype(np.float32, copy=False)

    # host-side O(B*D) prep
    fp64 = fp.astype(np.float64)
    norms2 = (fp64 * fp64).sum(axis=1)                  # [B]
    rowsum = fp64.sum(axis=1)                           # [B]
    r = 1.0 / np.maximum(np.sqrt(norms2), 1e-12)        # [B]
    r32 = r.astype(np.float32)

    # [K=128, 2, B] DoubleRow layout: D index = ktile*128 + partition.
    # mm8 (moving) carries r_i/T so psum holds sim/T directly.
    fT = np.ascontiguousarray(fp.T).reshape(2, P, B).transpose(1, 0, 2)
    sT = (r32 / np.float32(TEMPERATURE)).astype(np.float32)
    m8_full = np.ascontiguousarray(fT * sT[None, None, :]).astype(
        ml_dtypes.float8_e4m3)
    a8 = np.ascontiguousarray(fT * r32[None, None, :]).astype(
        ml_dtypes.float8_e4m3)

    # label selector: sel[p, q, kt, c] = 1 iff label(j=256q+128kt+p) == c
    lab = (np.arange(B) >= n0)
    labq = lab.reshape(NPAIR, 2, P)
    sel = np.zeros((P, NPAIR, 2, 32), dtype=ml_dtypes.float8_e5m2)
    sel[:, :, :, 0] = (~labq).transpose(2, 0, 1).astype(ml_dtypes.float8_e5m2)
    sel[:, :, :, 1] = labq.transpose(2, 0, 1).astype(ml_dtypes.float8_e5m2)

    # SW-interleaved stationary layout for DoubleRowSwInterleave: per block
    # and partition the 256 weights are stored A127 B127 A126 B126 ... B0
    # (A = k-tile 0, B = k-tile 1, columns reversed).
    a8i = np.ascontiguousarray(
        a8.reshape(P, 2, NBLK, P)[:, :, :, ::-1].transpose(0, 2, 3, 1)
        .reshape(P, NBLK, 256))

    nc = _get_program()
    in_maps = []
    for c in range(NCORES):
        sl = slice(c * BPC, (c + 1) * BPC)
        in_maps.append({"a8": a8i, "sel": sel,
                        "mm8": np.ascontiguousarray(m8_full[:, :, sl])})
    res = run_bass_kernel_spmd(nc, in_maps, list(range(NCORES)), trace=trace)

    parts = [res.results[c]["stats"].astype(np.float64) for c in range(NCORES)]
    S0 = np.concatenate([p[0] for p in parts]) * (2.0 ** SCALE_LOG)
    S1 = np.concatenate([p[1] for p in parts]) * (2.0 ** SCALE_LOG)

    # Diagonal exp reproduction: q'[i] = the device's own quantized
    # self-product (f64); formula matched to the engine of block i//128.
    q = np.einsum("pkj,pkj->j", m8_full.astype(np.float64),
                  a8.astype(np.float64))
    act_block = ((np.arange(B) // P) % 2) == 0
    dd = np.where(act_block, _actexp_f64(q), _schraud_f64(q))

    fn64 = fp64 * r[:, None]
    stats = {"norms2": norms2, "rowsum": rowsum, "S0": S0, "S1": S1,
             "d": dd, "fn64": fn64}
    return stats, order, n0, res


def _contrastive(stats, order, n0, labels, B):
    """Per-row r_con (f64) from device sums + host safety net."""
    labels_p = labels[order]
    nmf = (labels_p == 0)
    S0 = stats["S0"]
    S1 = stats["S1"]
    ddiag = stats["d"]

    S_same = np.where(nmf, S0, S1)
    S_diff = np.where(nmf, S1, S0)
    pos_sum = S_same - ddiag
    neg_sum = S_diff.copy()
    n1 = B - n0
    cnt_pos = np.where(nmf, n0 - 1, n1 - 1)
    cnt_neg = np.where(nmf, n1, n0)
    has_both = (cnt_pos > 0) & (cnt_neg > 0)

    if has_both.any():
        hb_pos = pos_sum[has_both]
        med = np.median(hb_pos[np.isfinite(hb_pos)]) if np.isfinite(
            hb_pos).any() else 0.0
        thresh = 0.25 * max(med, 0.0)
        suspect = has_both & (
            (pos_sum < thresh) | ~np.isfinite(pos_sum)
            | (neg_sum <= 0.0) | ~np.isfinite(neg_sum))
        idx = np.nonzero(suspect)[0]
        if idx.size:
            fn = stats["fn64"]
            for lo in range(0, idx.size, 512):
                rows = idx[lo:lo + 512]
                sims = (fn[rows] @ fn.T) / TEMPERATURE
                e = np.exp(sims)
                same = labels_p[rows][:, None] == labels_p[None, :]
                e_same = np.where(same, e, 0.0).sum(axis=1)
                e_diff = np.where(same, 0.0, e).sum(axis=1)
                self_e = np.exp(sims[np.arange(rows.size), rows])
                pos_sum[rows] = e_same - self_e
                neg_sum[rows] = e_diff

    pos_safe = np.where(has_both, np.maximum(pos_sum, 1e-12), 1.0)
    den_safe = np.where(has_both, pos_sum + neg_sum + 1e-8, 1.0)
    return np.where(has_both, -np.log(pos_safe / den_safe), 0.0), labels_p


def finalize(stats, order, n0, labels, normal_center, running_sigma, B):
    """Host O(B) finalization mirroring the reference formulas (float64)."""
    r_con, labels_p = _contrastive(stats, order, n0, labels, B)
    nmf = (labels_p == 0)
    amf = (labels_p == 1)
    norms2 = stats["norms2"]
    rowsum = stats["rowsum"]

    c = np.asarray(normal_center, dtype=np.float64)
    csq = float((c * c).sum())
    if csq != 0.0:
        raise NotImplementedError  # caller routes to the general-center path
    dist_sq = norms2  # center == 0
    n_normal = float(nmf.sum())

    with np.errstate(divide="ignore", invalid="ignore"):
        n_el = n_normal * D
        masked_sum = float((rowsum * nmf).sum())
        mean = masked_sum / n_el
        sum_sq_m = float((norms2 * nmf).sum())
        var = (sum_sq_m - 2.0 * mean * masked_sum + mean * mean * n_el) / (n_el - 1.0)
        sigma_new = 0.9 * float(running_sigma) + 0.1 * np.sqrt(var)

        m_adaptive = (MARGIN_BASE + LAMBDA_SIGMA * sigma_new
                      + LAMBDA_RESOLUTION * (1.0 - RESOLUTION_RATIO))
        dist = np.sqrt(np.maximum(dist_sq, 0.0))
        r_center = dist_sq * nmf
        r_margin = np.maximum(m_adaptive - dist, 0.0) * amf

        raw_total = ALPHA * r_center + BETA * r_margin + GAMMA * r_con
        total = raw_total.mean()
    return np.array(total, dtype=np.float32)


def _finalize_general_center(stats, order, n0, labels, normal_center,
                             running_sigma, B, features):
    """Fallback for a nonzero normal_center (not hit for spec inputs)."""
    r_con, labels_p = _contrastive(stats, order, n0, labels, B)
    fp = features[order].astype(np.float64)
    c = np.asarray(normal_center, dtype=np.float64)
    qc = fp @ c
    norms2 = stats["norms2"]
    dist_sq = norms2 - 2.0 * qc + float((c * c).sum())
    nmf = (labels_p == 0)
    amf = (labels_p == 1)
    rowsum = stats["rowsum"]
    n_normal = float(nmf.sum())
    with np.errstate(divide="ignore", invalid="ignore"):
        n_el = n_normal * D
        masked_sum = float((rowsum * nmf).sum())
        mean = masked_sum / n_el
        sum_sq_m = float((norms2 * nmf).sum())
        var = (sum_sq_m - 2.0 * mean * masked_sum + mean * mean * n_el) / (n_el - 1.0)
        sigma_new = 0.9 * float(running_sigma) + 0.1 * np.sqrt(var)
        m_adaptive = (MARGIN_BASE + LAMBDA_SIGMA * sigma_new
                      + LAMBDA_RESOLUTION * (1.0 - RESOLUTION_RATIO))
        dist = np.sqrt(np.maximum(dist_sq, 0.0))
        r_center = dist_sq * nmf
        r_margin = np.maximum(m_adaptive - dist, 0.0) * amf
        total = (ALPHA * r_center + BETA * r_margin + GAMMA * r_con).mean()
    return np.array(total, dtype=np.float32)


def kernel(features, labels, normal_center, running_sigma):
    features = np.asarray(features, dtype=np.float32)
    labels = np.asarray(labels, dtype=np.int32)
    normal_center = np.asarray(normal_center, dtype=np.float32)
    running_sigma = np.float32(np.asarray(running_sigma))
    Bq = features.shape[0]

    stats, order, n0, _res = run_device(features, labels)
    if float((np.asarray(normal_center, np.float64) ** 2).sum()) != 0.0:
        return _finalize_general_center(stats, order, n0, labels,
                                        normal_center, running_sigma, Bq,
                                        features)
    return finalize(stats, order, n0, labels, normal_center, running_sigma, Bq)


# revision 20
# speedup vs baseline: 1.0598x; 1.0126x over previous
"""MACCL loss kernel for Trainium2 (8 NeuronCores, SPMD data-parallel).

Strategy (v4: transposed j-blocks, PE-side reduction)
-----------------------------------------------------
The O(B^2) contrastive exp/row-sum dominates.  v2 ran it all on the
Scalar (ACT) engine (~67us busy); v3's ACT/DVE column split stalled on
convoy effects (DVE needs 2 passes row-major, PE ran cold).  v4 flips
the orientation: each core computes sim^T in 64 j-blocks of 128
columns; per block the stationary operand is the a8 j-block and the
moving operand is the core's own 1024 rows, so psum holds [128 j-rows,
1024 i-cols].

  - Even blocks -> ACT: true exp, activation(psum, Exp, bias=-7ln2),
    output straight to fp8(e5m2) scratch (exp(q)*2^-7), NO accumulator.
  - Odd blocks -> DVE: Schraudolph fast exp in ONE 1x pass:
    n = int8(psum*C1E + C2E); the int8 bits ARE the e5m2 encoding of
    exp(q)*2^-7.
  - The row sums (pos/neg by label) are done by the PE itself: per
    block pair two tiny fp8 DoubleRow matmuls with a [256 x 32] label
    selector (only cols 0/1 nonzero; labels ride in as DATA, so one
    program serves any n0) accumulate S0/S1 for all 1024 rows into two
    PSUM banks (one per 512-row i-half; DR dst must sit at the col_grp
    base partition).  Dummy matmuls during the DMA lead-in pre-warm the
    PE's HAM clock gate.

All engines stream continuously (PE stays HAM-warm), there are no
ACT read-accumulator ops, and the only DVE work is one 1x pass per odd
block.  Host subtracts the huge exp(diag) (~1.6e6 vs pos_sum ~6.5e3)
with a bit-exact f64 replication of whichever engine produced that
column (np.exp+e5m2 rounding for ACT blocks, the int8 Schraudolph for
DVE blocks); a safety net recomputes exactly any row whose pos/neg
sums look corrupted (rare fp8 rounding flips / low-tail underflow), so
numerics are robust to convert rounding modes.

Host prep (O(B*D), f64): label-sort rows, row norms/sums for the
center/margin/sigma terms, fp8(e4m3) quantization of both operands
(stationary carries r_j, moving carries r_i/T).
"""

import os
import sys

for _p in ("/root/.axon_site", "/root/.axon_site/_ro/trn_rl_repo",
           "/root/.axon_site/_ro/pypackages", "/opt/trn_rl_repo", "/opt/pypackages"):
    if os.path.isdir(_p) and _p not in sys.path:
        sys.path.append(_p)

import numpy as np
import ml_dtypes
from contextlib import ExitStack

import concourse.bass as bass
import concourse.bacc as bacc
import concourse.tile as tile
from concourse import mybir
from concourse.bass_utils import run_bass_kernel_spmd

F32 = mybir.dt.float32
I8 = mybir.dt.int8
F8 = mybir.dt.float8e4
F8E5 = mybir.dt.float8e5

P = 128
D = 256
B = 8192
NCORES = 8
BPC = B // NCORES          # 1024 rows per core
NBLK = B // P              # 64 j-blocks
NPAIR = NBLK // 2          # 32 block pairs
N_MM = 512
TEMPERATURE = 0.07
MARGIN_BASE = 0.5
LAMBDA_SIGMA = 0.3
LAMBDA_RESOLUTION = 0.3
RESOLUTION_RATIO = 224.0 / 900.0
ALPHA, BETA, GAMMA = 1.0, 1.0, 0.5

LOG2 = float(np.log(2.0))
SCALE_LOG = 7                      # scratch holds exp(q) * 2^-SCALE_LOG
EBIAS = float(np.float32(-SCALE_LOG * LOG2))
# e5m2/int8 Schraudolph: n = rint(q*C1E + C2E); bitcast_e5m2(n) ~ exp(q)*2^-7
C1E = float(np.float32(4.0 / LOG2))
C2E = float(np.float32(32.0 - 0.22))


def build_program():
    AF = mybir.ActivationFunctionType
    ALU = mybir.AluOpType
    DR = mybir.MatmulPerfMode.DoubleRow

    nc = bacc.Bacc("TRN2", target_bir_lowering=False, debug=False,
                   num_devices=NCORES)
    a8_d = nc.dram_tensor("a8", [P, 2, B], F8, kind="ExternalInput").ap()
    mm8_d = nc.dram_tensor("mm8", [P, 2, BPC], F8, kind="ExternalInput").ap()
    sel_d = nc.dram_tensor("sel", [P, NPAIR, 2, 32], F8E5,
                           kind="ExternalInput").ap()
    stats_d = nc.dram_tensor("stats", [2, BPC], F32, kind="ExternalOutput").ap()

    with tile.TileContext(nc) as tc, ExitStack() as ctx:
        singles = ctx.enter_context(tc.tile_pool(name="singles", bufs=1))
        ps_pool = ctx.enter_context(tc.tile_pool(name="ps", bufs=1, space="PSUM"))

        a8_sb = singles.tile([P, 2, B], F8)
        m8_sb = singles.tile([P, 2, BPC], F8)
        sel_sb = singles.tile([P, NPAIR, 2, 32], F8E5)
        scrs = [singles.tile([P, 2, BPC], F8E5, name=f"scr{t}")
                for t in range(3)]
        stats_sb = singles.tile([P, BPC], F32)
        prime = singles.tile([P, 1], F32)
        ebias_t = singles.tile([P, 1], F32)
        nc.vector.memset(ebias_t, EBIAS)

        # Priming activation with no input deps (scale=0 ignores the garbage
        # read): hoists the ~1.5us ACT table load into the DMA lead-in.
        nc.scalar.activation(prime, prime, AF.Exp, scale=0.0)

        # moving operand + selectors first (gate everything), then a8 with a
        # fine-grained head so the first matmuls start early.
        nc.sync.dma_start(m8_sb[:, :, 0:N_MM], mm8_d[:, :, 0:N_MM])
        nc.sync.dma_start(a8_sb[:, :, 0:P], a8_d[:, :, 0:P])
        nc.sync.dma_start(m8_sb[:, :, N_MM:BPC], mm8_d[:, :, N_MM:BPC])
        # a8 streams on the sync queue, fine-grained at the head so the
        # first blocks start early; the selector DMA is placed after the
        # first few blocks but well before the first selector matmul.
        cuts = [128, 256, 512, 1024, 2048, 3072, 4096, 5120, 6144, 7168, B]
        for ci, (c0, c1) in enumerate(zip(cuts[:-1], cuts[1:])):
            nc.sync.dma_start(a8_sb[:, :, c0:c1], a8_d[:, :, c0:c1])
            if ci == 0:
                nc.sync.dma_start(sel_sb, sel_d)

        Ts = [ps_pool.tile([P, BPC], F32, name=f"T{t}") for t in range(3)]
        accs = [ps_pool.tile([P, N_MM], F32, name=f"acc{h}") for h in (0, 1)]

        # HAM warm-up: keep the PE busy during the DMA lead-in (on the first
        # m8 chunk, which lands early) so the 4096-cycle activity window
        # un-throttles the clock before the real matmul stream begins.
        for w in range(14):
            nc.tensor.matmul(Ts[2][:, 0:64], m8_sb[:, :, 0:P],
                             m8_sb[:, :, 0:64], start=True, stop=True,
                             perf_mode=DR)

        def emit_sel(q):
            sc = scrs[q % 3]
            for h in (0, 1):
                nc.tensor.matmul(accs[h][0:32, :],
                                 sel_sb[:, q, :, :],
                                 sc[:, :, h * N_MM:(h + 1) * N_MM],
                                 start=(q == 0), stop=(q == NPAIR - 1),
                                 perf_mode=DR, skip_group_check=True)

        for b in range(NBLK):
            q, half = divmod(b, 2)
            if half == 0 and q >= 2:
                emit_sel(q - 2)
            T = Ts[b % 3]
            lhsT = a8_sb[:, :, b * P:(b + 1) * P]
            for h in (0, 1):
                nc.tensor.matmul(T[:, h * N_MM:(h + 1) * N_MM], lhsT,
                                 m8_sb[:, :, h * N_MM:(h + 1) * N_MM],
                                 start=True, stop=True, perf_mode=DR)
            sc = scrs[q % 3]
            if half == 0:
                nc.scalar.activation(sc[:, 0, :], T, AF.Exp,
                                     bias=ebias_t[:, 0:1])
            else:
                nc.vector.tensor_scalar(sc.bitcast(I8)[:, 1, :], T,
                                        C1E, C2E, ALU.mult, ALU.add)
        emit_sel(NPAIR - 2)
        emit_sel(NPAIR - 1)

        # acc_h rows 0/1 = S0/S1 for i-half h.
        nc.vector.tensor_copy(stats_sb[0:2, 0:N_MM], accs[0][0:2, :])
        nc.scalar.copy(stats_sb[0:2, N_MM:BPC], accs[1][0:2, :])
        nc.sync.dma_start(stats_d[:, 0:N_MM], stats_sb[0:2, 0:N_MM])
        nc.sync.dma_start(stats_d[:, N_MM:BPC], stats_sb[0:2, N_MM:BPC])

    nc.compile()
    return nc


_PROGRAM_CACHE = {}


def _get_program():
    if "p" not in _PROGRAM_CACHE:
        _PROGRAM_CACHE["p"] = build_program()
    return _PROGRAM_CACHE["p"]


def _schraud_f64(q):
    """f64 replication of the device DVE fast exp (RNE convert assumed),
    including the 2^SCALE_LOG unscale."""
    y1 = (q.astype(np.float32) * np.float32(C1E)).astype(np.float32)
    y2 = (y1 + np.float32(C2E)).astype(np.float32)
    n = np.rint(y2.astype(np.float64)).astype(np.int8)
    return n.view(ml_dtypes.float8_e5m2).astype(np.float64) * (2.0 ** SCALE_LOG)


def _actexp_f64(q):
    """f64 replication of the ACT-block diag: exp(q-7ln2) rounded to e5m2."""
    v = np.exp(q - SCALE_LOG * LOG2).astype(np.float32)
    v8 = v.astype(ml_dtypes.float8_e5m2)
    return v8.astype(np.float64) * (2.0 ** SCALE_LOG)


def run_device(features, labels, trace=False):
    """Host prep + 8-core device run.  Returns (stats dict aligned to the
    label-sorted permutation, permutation order, n0, raw results)."""
    Bq, d = features.shape
    assert d == D and Bq == B

    order = np.argsort(labels, kind="stable")
    n0 = int((labels == 0).sum())
    fp = np.ascontiguousarray(features[order]).astype(np.float32, copy=False)

    # host-side O(B*D) prep
    fp64 = fp.astype(np.float64)
    norms2 = (fp64 * fp64).sum(axis=1)                  # [B]
    rowsum = fp64.sum(axis=1)                           # [B]
    r = 1.0 / np.maximum(np.sqrt(norms2), 1e-12)        # [B]
    r32 = r.astype(np.float32)

    # [K=128, 2, B] DoubleRow layout: D index = ktile*128 + partition.
    # mm8 (moving) carries r_i/T so psum holds sim/T directly.
    fT = np.ascontiguousarray(fp.T).reshape(2, P, B).transpose(1, 0, 2)
    sT = (r32 / np.float32(TEMPERATURE)).astype(np.float32)
    m8_full = np.ascontiguousarray(fT * sT[None, None, :]).astype(
        ml_dtypes.float8_e4m3)
    a8 = np.ascontiguousarray(fT * r32[None, None, :]).astype(
        ml_dtypes.float8_e4m3)

    # label selector: sel[p, q, kt, c] = 1 iff label(j=256q+128kt+p) == c
    lab = (np.arange(B) >= n0)
    labq = lab.reshape(NPAIR, 2, P)
    sel = np.zeros((P, NPAIR, 2, 32), dtype=ml_dtypes.float8_e5m2)
    sel[:, :, :, 0] = (~labq).transpose(2, 0, 1).astype(ml_dtypes.float8_e5m2)
    sel[:, :, :, 1] = labq.transpose(2, 0, 1).astype(ml_dtypes.float8_e5m2)

    nc = _get_program()
    in_maps = []
    for c in range(NCORES):
        sl = slice(c * BPC, (c + 1) * BPC)
        in_maps.append({"a8": a8, "sel": sel,
                        "mm8": np.ascontiguousarray(m8_full[:, :, sl])})
    res = run_bass_kernel_spmd(nc, in_maps, list(range(NCORES)), trace=trace)

    parts = [res.results[c]["stats"].astype(np.float64) for c in range(NCORES)]
    S0 = np.concatenate([p[0] for p in parts]) * (2.0 ** SCALE_LOG)
    S1 = np.concatenate([p[1] for p in parts]) * (2.0 ** SCALE_LOG)

    # Diagonal exp reproduction: q'[i] = the device's own quantized
    # self-product (f64); formula matched to the engine of block i//128.
    q = np.einsum("pkj,pkj->j", m8_full.astype(np.float64),
                  a8.astype(np.float64))
    act_block = ((np.arange(B) // P) % 2) == 0
    dd = np.where(act_block, _actexp_f64(q), _schraud_f64(q))

    fn64 = fp64 * r[:, None]
    stats = {"norms2": norms2, "rowsum": rowsum, "S0": S0, "S1": S1,
             "d": dd, "fn64": fn64}
    return stats, order, n0, res


def _contrastive(stats, order, n0, labels, B):
    """Per-row r_con (f64) from device sums + host safety net."""
    labels_p = labels[order]
    nmf = (labels_p == 0)
    S0 = stats["S0"]
    S1 = stats["S1"]
    ddiag = stats["d"]

    S_same = np.where(nmf, S0, S1)
    S_diff = np.where(nmf, S1, S0)
    pos_sum = S_same - ddiag
    neg_sum = S_diff.copy()
    n1 = B - n0
    cnt_pos = np.where(nmf, n0 - 1, n1 - 1)
    cnt_neg = np.where(nmf, n1, n0)
    has_both = (cnt_pos > 0) & (cnt_neg > 0)

    if has_both.any():
        hb_pos = pos_sum[has_both]
        med = np.median(hb_pos[np.isfinite(hb_pos)]) if np.isfinite(
            hb_pos).any() else 0.0
        thresh = 0.25 * max(med, 0.0)
        suspect = has_both & (
            (pos_sum < thresh) | ~np.isfinite(pos_sum)
            | (neg_sum <= 0.0) | ~np.isfinite(neg_sum))
        idx = np.nonzero(suspect)[0]
        if idx.size:
            fn = stats["fn64"]
            for lo in range(0, idx.size, 512):
                rows = idx[lo:lo + 512]
                sims = (fn[rows] @ fn.T) / TEMPERATURE
                e = np.exp(sims)
                same = labels_p[rows][:, None] == labels_p[None, :]
                e_same = np.where(same, e, 0.0).sum(axis=1)
                e_diff = np.where(same, 0.0, e).sum(axis=1)
                self_e = np.exp(sims[np.arange(rows.size), rows])
                pos_sum[rows] = e_same - self_e
                neg_sum[rows] = e_diff

    pos_safe = np.where(has_both, np.maximum(pos_sum, 1e-12), 1.0)
    den_safe = np.where(has_both, pos_sum + neg_sum + 1e-8, 1.0)
    return np.where(has_both, -np.log(pos_safe / den_safe), 0.0), labels_p


def finalize(stats, order, n0, labels, normal_center, running_sigma, B):
    """Host O(B) finalization mirroring the reference formulas (float64)."""
    r_con, labels_p = _contrastive(stats, order, n0, labels, B)
    nmf = (labels_p == 0)
    amf = (labels_p == 1)
    norms2 = stats["norms2"]
    rowsum = stats["rowsum"]

    c = np.asarray(normal_center, dtype=np.float64)
    csq = float((c * c).sum())
    if csq != 0.0:
        raise NotImplementedError  # caller routes to the general-center path
    dist_sq = norms2  # center == 0
    n_normal = float(nmf.sum())

    with np.errstate(divide="ignore", invalid="ignore"):
        n_el = n_normal * D
        masked_sum = float((rowsum * nmf).sum())
        mean = masked_sum / n_el
        sum_sq_m = float((norms2 * nmf).sum())
        var = (sum_sq_m - 2.0 * mean * masked_sum + mean * mean * n_el) / (n_el - 1.0)
        sigma_new = 0.9 * float(running_sigma) + 0.1 * np.sqrt(var)

        m_adaptive = (MARGIN_BASE + LAMBDA_SIGMA * sigma_new
                      + LAMBDA_RESOLUTION * (1.0 - RESOLUTION_RATIO))
        dist = np.sqrt(np.maximum(dist_sq, 0.0))
        r_center = dist_sq * nmf
        r_margin = np.maximum(m_adaptive - dist, 0.0) * amf

        raw_total = ALPHA * r_center + BETA * r_margin + GAMMA * r_con
        total = raw_total.mean()
    return np.array(total, dtype=np.float32)


def _finalize_general_center(stats, order, n0, labels, normal_center,
                             running_sigma, B, features):
    """Fallback for a nonzero normal_center (not hit for spec inputs)."""
    r_con, labels_p = _contrastive(stats, order, n0, labels, B)
    fp = features[order].astype(np.float64)
    c = np.asarray(normal_center, dtype=np.float64)
    qc = fp @ c
    norms2 = stats["norms2"]
    dist_sq = norms2 - 2.0 * qc + float((c * c).sum())
    nmf = (labels_p == 0)
    amf = (labels_p == 1)
    rowsum = stats["rowsum"]
    n_normal = float(nmf.sum())
    with np.errstate(divide="ignore", invalid="ignore"):
        n_el = n_normal * D
        masked_sum = float((rowsum * nmf).sum())
        mean = masked_sum / n_el
        sum_sq_m = float((norms2 * nmf).sum())
        var = (sum_sq_m - 2.0 * mean * masked_sum + mean * mean * n_el) / (n_el - 1.0)
        sigma_new = 0.9 * float(running_sigma) + 0.1 * np.sqrt(var)
        m_adaptive = (MARGIN_BASE + LAMBDA_SIGMA * sigma_new
                      + LAMBDA_RESOLUTION * (1.0 - RESOLUTION_RATIO))
        dist = np.sqrt(np.maximum(dist_sq, 0.0))
        r_center = dist_sq * nmf
        r_margin = np.maximum(m_adaptive - dist, 0.0) * amf
        total = (ALPHA * r_center + BETA * r_margin + GAMMA * r_con).mean()
    return np.array(total, dtype=np.float32)


def kernel(features, labels, normal_center, running_sigma):
    features = np.asarray(features, dtype=np.float32)
    labels = np.asarray(labels, dtype=np.int32)
    normal_center = np.asarray(normal_center, dtype=np.float32)
    running_sigma = np.float32(np.asarray(running_sigma))
    Bq = features.shape[0]

    stats, order, n0, _res = run_device(features, labels)
    if float((np.asarray(normal_center, np.float64) ** 2).sum()) != 0.0:
        return _finalize_general_center(stats, order, n0, labels,
                                        normal_center, running_sigma, Bq,
                                        features)
    return finalize(stats, order, n0, labels, normal_center, running_sigma, Bq)
